# revision 1
# baseline (speedup 1.0000x reference)
"""MoE top-1 feed-forward (DeepSpeed-style) on 8 Trainium2 NeuronCores.

Strategy (expert parallelism, per the sharding hint):
  - Host computes the (tiny) gate: logits = x @ Wg, softmax, top-1 expert id
    and gate prob per token (float64 for a faithful argmax).
  - Tokens are dispatched to the core owning their expert (core e holds
    W1[e]/b1[e]/W2[e]/b2[e]); each core's token batch is padded to a common
    capacity C so all 8 cores run one SPMD program.
  - Each core runs the dense FFN for its tokens:
        hT = silu(W1^T @ xT + b1);  yT = W2^T @ hT
    with tokens laid out along the free (moving) dimension so no transposes
    are needed on device: xT is [D, C], hT is [F, C], yT is [D, C].
  - All weights are SBUF-resident (~75KB/partition in bf16) and their DMAs
    are issued eagerly up-front, striped over the three DMA-capable rings
    (SP / ACT / Pool) in consumption order. Measured on HW: no ring moves
    data until ~9-10us after kernel start and each queue tops out around
    80-135 GB/s, so all three must stream continuously; the PE start is
    gated on sentinel tiles so that once running it never stalls (a stall
    also costs a ~1.5us p-state re-ramp).
  - mm2 trails mm1 by four chunks (PSUM-resident y accumulators), so late
    W2 arrivals don't bubble the PE; the tail evacuates PSUM on DVE+ACT
    casting to bf16 and stores via two DMAs on the idle SP/ACT rings.
  - Host combines: out[token] = gate * (y + b2[expert]).
"""

import os
import sys

import numpy as np

try:
    import concourse.mybir as mybir  # noqa: F401
except ModuleNotFoundError:  # fallback if the site hooks aren't installed
    sys.path.insert(0, "/opt/trn_rl_repo")

import concourse.mybir as mybir
import concourse.tile as tile
from concourse import bacc
from concourse.bass_utils import run_bass_kernel_spmd

N_CORES = 8

# Compute dtype for the matmuls:
#   "bf16" - weights/activations cast to bfloat16 (f32 PSUM accumulate).
#            Same 1 cycle/row PE rate as f32r but half the HBM traffic,
#            which is what this kernel is limited by.
#   "f32r" - fp32 data, PE's replicated-fp32 mode (full rate at N>=256)
#   "f32"  - plain fp32 matmuls (4x slower PE)
MODE = os.environ.get("BASS_MOE_MODE", "bf16")

FG = int(os.environ.get("BASS_MOE_FG", "2"))  # steady-state f-chunks per W1 group
W2P = int(os.environ.get("BASS_MOE_W2P", "2"))  # f-chunks per W2 pair-tile


def _w1_groups(KF):
    """F-chunk widths per W1 group: small leading groups let the PE start
    before a whole FG-wide image lands, and small steady-state groups keep
    each DMA under ~400KB so the ~120GB/s-per-queue rings interleave finely."""
    lead = [1, 1] if KF > 8 and FG >= 2 else ([2, 2] if FG > 2 and KF > 4 else [])
    rem = KF - sum(lead)
    groups = list(lead)
    while rem > 0:
        w = min(FG, rem)
        groups.append(w)
        rem -= w
    return groups


_CACHE: dict = {}


def _roundup(a: int, m: int) -> int:
    return -(-a // m) * m


def _build_bass(C: int, n_slabs: int, mode: str, D: int, F: int):
    """Build + compile the per-core Bass program for capacity C (divisible by
    n_slabs; slab width CS = C/n_slabs must be 256..512)."""
    f32 = mybir.dt.float32
    if mode == "bf16":
        dt_io = mybir.dt.bfloat16
    elif mode == "f32r":
        dt_io = mybir.dt.float32r
    else:
        dt_io = f32

    KD, KF = D // 128, F // 128
    GRPS = _w1_groups(KF)
    NP = KF // W2P  # number of W2 pair-tiles
    CS = C // n_slabs
    assert C % n_slabs == 0 and 256 <= CS <= 512

    nc = bacc.Bacc(None, target_bir_lowering=False, debug=False)
    # Host-packed images (see kernel() for the packing):
    #   xT   [128, KD*C]             col d*C+t = x^T[d*128+p, t]
    #   w1   [128, KD*F]             flat group images; group g at column
    #                                offset KD*128*sum(GRPS[:g]), blocks (d, j)
    #                                within a group at (d*gw+j)*128
    #   w2   [NP, 128, W2P*D]        w2[p] f-chunk r=f-p*W2P at cols r*D
    #   b1r  [128, KF]               b1[f*128+p] at [p, f]
    #   yT   [128, KD*C]             output, same layout as xT (dt_io)
    HX = KD // 2
    xA = nc.dram_tensor("xA", [128, HX * C], dt_io, kind="ExternalInput")
    xB = nc.dram_tensor("xB", [128, (KD - HX) * C], dt_io, kind="ExternalInput")
    w1 = nc.dram_tensor("w1", [128 * KD * F], dt_io, kind="ExternalInput")
    w2 = nc.dram_tensor("w2", [NP, 128, W2P * D], dt_io, kind="ExternalInput")
    b1r = nc.dram_tensor("b1r", [128, KF], f32, kind="ExternalInput")
    yA = nc.dram_tensor("yA", [128, HX * C], dt_io, kind="ExternalOutput")
    yB = nc.dram_tensor("yB", [128, (KD - HX) * C], dt_io, kind="ExternalOutput")

    silu = mybir.ActivationFunctionType.Silu

    with tile.TileContext(nc) as tc:
        with (
            tc.tile_pool(name="xp", bufs=1) as xp,
            tc.tile_pool(name="wp", bufs=1) as wp,
            tc.tile_pool(name="hp", bufs=6) as hp,
            tc.tile_pool(name="bp", bufs=1) as bp,
            tc.tile_pool(name="yp", bufs=2) as yp,
            tc.tile_pool(name="ps_h", bufs=2, space="PSUM") as ps_h,
            tc.tile_pool(name="ps_y", bufs=1, space="PSUM") as ps_y,
        ):
            # ---- tiles ----
            b1t = bp.tile([128, KF], f32, tag="b1", name="b1t")
            # x arrives as two half-width images on different rings so the
            # first mm1 isn't gated on one queue moving the whole 0.4MB
            xwa = [
                xp.tile([128, HX * CS], dt_io, tag=f"xwa{s}", name=f"xwa{s}")
                for s in range(n_slabs)
            ]
            xwb = [
                xp.tile([128, (KD - HX) * CS], dt_io, tag=f"xwb{s}", name=f"xwb{s}")
                for s in range(n_slabs)
            ]
            w1ts = []
            f0 = 0
            w1_offs = []
            for g, gw in enumerate(GRPS):
                w1ts.append(
                    wp.tile([128, KD * gw * 128], dt_io, tag=f"w1_{g}", name=f"w1t{g}")
                )
                w1_offs.append(f0)
                f0 += gw
            w2ts = [
                wp.tile([128, W2P * D], dt_io, tag=f"w2_{p}", name=f"w2t{p}")
                for p in range(NP)
            ]

            def load_w1(eng, g):
                # each group is a contiguous partition-major DRAM block so
                # the SDMA merges partition lines into large packets (the
                # queues are packet-rate bound)
                o = 128 * KD * 128 * w1_offs[g]
                w = KD * GRPS[g] * 128
                eng.dma_start(
                    out=w1ts[g][:],
                    in_=w1[o : o + 128 * w].rearrange("(p w) -> p w", p=128),
                )

            def load_w2(eng, p):
                eng.dma_start(out=w2ts[p][:], in_=w2[p])

            # ---- load scheduling ----
            # Only SP / ACT / Pool can initiate DMAs; no ring moves data until
            # ~9-10us after kernel start (DGE spin-up) and EACH queue tops out
            # around 110-130 GB/s regardless of HWDGE/SWDGE, so all three
            # queues must stream continuously. Items are striped across the
            # rings with a greedy earliest-completion schedule (computed
            # against the measured queue starts/rates) so every tile lands a
            # few us before the PE consumes it.
            ng = len(GRPS)
            if ng == 13 and NP == 12:
                # Stripe at measured queue rates (sync ~100, act ~80, pool
                # ~135 GB/s; first data ~8.5/9/9.5us). The PE start is gated
                # on g2's arrival (~14.4us) so that from there on every tile
                # lands before the PE needs it and the PE runs one unbroken
                # stretch - each stall also costs a ~1.5us p-state re-ramp,
                # so stalls are doubly expensive.
                sync_items = [("g", 1), ("g", 3), ("g", 5), ("p", 5), ("p", 7),
                              ("g", 9), ("p", 8), ("p", 11)]
                act_pre = [("p", 0), ("p", 2), ("p", 3)]
                act_mid = [(4, "p", 6), (8, "p", 9)]
                pool_items = [("g", 0), ("g", 2), ("g", 4), ("p", 1), ("g", 6),
                              ("g", 7), ("p", 4), ("g", 8), ("g", 10), ("g", 11),
                              ("g", 12), ("p", 10)]
                gate_tiles = (2, 3)  # w1 groups whose arrival releases the x fanout
            else:  # generic fallback (not tuned)
                sync_items = [("g", g) for g in range(ng)]
                act_pre = [("p", p) for p in range(min(2, NP))]
                act_mid = []
                pool_items = [("p", p) for p in range(min(2, NP), NP)]
                gate_tiles = ()

            def load(eng, kind, i):
                (load_w1 if kind == "g" else load_w2)(eng, i)

            xAv = xA.rearrange("p (k c) -> p k c", k=HX)
            xBv = xB.rearrange("p (k c) -> p k c", k=KD - HX)

            def load_x(s):
                # the halves are contiguous DRAM blocks: one wide packet run
                if n_slabs == 1:
                    nc.sync.dma_start(out=xwa[0][:], in_=xA[:])
                    nc.scalar.dma_start(out=xwb[0][:], in_=xB[:])
                else:
                    nc.sync.dma_start(
                        out=xwa[s][:], in_=xAv[:, :, s * CS : (s + 1) * CS]
                    )
                    nc.scalar.dma_start(
                        out=xwb[s][:], in_=xBv[:, :, s * CS : (s + 1) * CS]
                    )

            # sync: first w1 group leads, then xwa - the x arrival (plus the
            # DVE fanout) is what releases the first mm1, so its ring
            # position paces the PE start without any reorderable gate op
            nc.scalar.dma_start(out=b1t[:], in_=b1r[:])
            load_x(0)
            for kind, i in sync_items:
                load(nc.sync, kind, i)
            for kind, i in act_pre:
                load(nc.scalar, kind, i)
            for kind, i in pool_items:
                load(nc.gpsimd, kind, i)
            for s in range(1, n_slabs):
                load_x(s)

            for s in range(n_slabs):
                c0 = s * CS
                # fan the wide x images out to narrow per-d tiles on the
                # (otherwise idle) vector engine; narrow rhs tiles keep the
                # PE moving-operand read on its fast path
                xt = []
                for d in range(KD):
                    src = xwa[s] if d < HX else xwb[s]
                    off = d if d < HX else d - HX
                    t = xp.tile([128, CS], dt_io, tag=f"x{d}", name=f"xt{d}")
                    nc.vector.tensor_copy(t[:], src[:, off * CS : (off + 1) * CS])
                    xt.append(t)

                def xsl(d):
                    return xt[d][:]

                py = [
                    ps_y.tile([128, CS], f32, tag=f"y{dd}", name=f"py{dd}")
                    for dd in range(KD)
                ]

                def emit_mm2(f, ht):
                    # yT += W2[f-chunk, :]^T @ hT[f-chunk]
                    p, r = divmod(f, W2P)
                    for dd in range(KD):
                        nc.tensor.matmul(
                            py[dd][:],
                            w2ts[p][:, r * D + dd * 128 : r * D + (dd + 1) * 128],
                            ht[:],
                            start=(f == 0),
                            stop=(f == KF - 1),
                        )

                # mm2 is deferred four chunks behind mm1: chunk f's silu runs
                # while mm1(f+1) is on the PE, and the extra slots ride out
                # late W2 arrivals during the DMA-ring start seam.
                pend: list = []
                f0 = 0
                for g, gw in enumerate(GRPS):
                    for j in range(gw):
                        f = f0 + j
                        # hT[f-chunk] = silu(sum_d W1[d, f-chunk]^T @ xT[d] + b1)
                        ph = ps_h.tile([128, CS], f32, tag="hps", name="ph")
                        for d in range(KD):
                            nc.tensor.matmul(
                                ph[:],
                                w1ts[g][:, (d * gw + j) * 128 : (d * gw + j + 1) * 128],
                                xsl(d),
                                start=(d == 0),
                                stop=(d == KD - 1),
                            )
                        ht = hp.tile([128, CS], dt_io, tag="ht", name="ht")
                        nc.scalar.activation(ht[:], ph[:], silu, bias=b1t[:, f : f + 1])
                        if s == 0 and act_mid and act_mid[0][0] == f:
                            _, kind, i = act_mid.pop(0)
                            load(nc.scalar, kind, i)
                        pend.append((f, ht))
                        if len(pend) > 4:
                            emit_mm2(*pend.pop(0))
                    f0 += gw
                while pend:
                    emit_mm2(*pend.pop(0))

                # tail: evacuate PSUM on both DVE and ACT (casting to dt_io),
                # stream out in two DMAs on the SP and ACT rings
                yt = yp.tile([128, KD * CS], dt_io, tag="yt", name="yt")
                half = KD // 2
                for dd in range(KD):
                    if dd < half:
                        nc.vector.tensor_copy(
                            yt[:, dd * CS : (dd + 1) * CS], py[dd][:]
                        )
                    else:
                        nc.scalar.copy(yt[:, dd * CS : (dd + 1) * CS], py[dd][:])
                if n_slabs == 1:
                    nc.sync.dma_start(out=yA[:], in_=yt[:, 0 : half * CS])
                    nc.scalar.dma_start(out=yB[:], in_=yt[:, half * CS :])
                else:
                    yAv = yA.rearrange("p (k c) -> p k c", k=half)
                    yBv = yB.rearrange("p (k c) -> p k c", k=KD - half)
                    nc.sync.dma_start(
                        out=yAv[:, :, c0 : c0 + CS], in_=yt[:, 0 : half * CS]
                    )
                    nc.scalar.dma_start(
                        out=yBv[:, :, c0 : c0 + CS], in_=yt[:, half * CS :]
                    )

    nc.compile()
    return nc


def _get_bass(C: int, n_slabs: int, mode: str, D: int, F: int):
    key = (C, n_slabs, mode, D, F, FG, W2P)
    if key not in _CACHE:
        _CACHE[key] = _build_bass(C, n_slabs, mode, D, F)
    return _CACHE[key]


def _gate_host(x: np.ndarray, Wg: np.ndarray):
    """Top-1 gating in float64: returns (expert_idx [T], gate [T] f32)."""
    logits = x.astype(np.float64) @ Wg.astype(np.float64)
    m = logits.max(-1, keepdims=True)
    p = np.exp(logits - m)
    p /= p.sum(-1, keepdims=True)
    return p.argmax(-1), p.max(-1).astype(np.float32)


def _kernel_numpy(x, Wg, W1, b1, W2, b2):
    """Reference-equivalent fallback (host only)."""
    idx, gate = _gate_host(x, Wg)
    out = np.zeros_like(x)
    for e in range(W1.shape[0]):
        ids = np.nonzero(idx == e)[0]
        if ids.size == 0:
            continue
        h = x[ids] @ W1[e] + b1[e]
        h = h * (1.0 / (1.0 + np.exp(-h)))
        out[ids] = gate[ids, None] * (h @ W2[e] + b2[e])
    return out


def kernel(hidden_states, Wg, W1, b1, W2, b2):
    hidden_states = np.asarray(hidden_states)
    Wg = np.asarray(Wg, dtype=np.float32)
    W1 = np.asarray(W1, dtype=np.float32)
    b1 = np.asarray(b1, dtype=np.float32)
    W2 = np.asarray(W2, dtype=np.float32)
    b2 = np.asarray(b2, dtype=np.float32)

    orig_shape = hidden_states.shape
    D = orig_shape[-1]
    x = np.ascontiguousarray(hidden_states, dtype=np.float32).reshape(-1, D)
    E, _, F = W1.shape
    KD, KF = D // 128, F // 128

    if E != N_CORES or D % 128 != 0 or F % 128 != 0 or KF % FG != 0:
        return _kernel_numpy(x, Wg, W1, b1, W2, b2).reshape(orig_shape)

    idx, gate = _gate_host(x, Wg)
    order = np.argsort(idx, kind="stable")
    counts = np.bincount(idx, minlength=E)
    starts = np.concatenate([[0], np.cumsum(counts)])

    # Capacity: common padded token count per core. Slab width must be
    # 256..512 (PSUM bank limit / fp32r fast path).
    C = max(256, _roundup(int(counts.max()), 16))
    n_slabs = -(-C // 512)
    C = n_slabs * max(256, _roundup(-(-C // n_slabs), 16))

    mode = MODE
    np_io = np.float32
    if mode == "bf16":
        import ml_dtypes

        np_io = ml_dtypes.bfloat16

    nc = _get_bass(C, n_slabs, mode, D, F)

    NP = KF // W2P
    in_maps = []
    for e in range(E):
        ids = order[starts[e] : starts[e + 1]]
        xe = np.zeros((C, D), dtype=np.float32)
        xe[: ids.size] = x[ids]
        # pack per-core images (see _build_bass docstring)
        xTr = xe.reshape(C, KD, 128).transpose(2, 1, 0).reshape(128, KD * C)
        HX = KD // 2
        grps = _w1_groups(KF)
        w1e = W1[e].reshape(KD, 128, KF, 128)
        parts = []
        f0 = 0
        for gw in grps:
            blk = w1e[:, :, f0 : f0 + gw]  # [KD, 128, gw, 128]
            parts.append(blk.transpose(1, 0, 2, 3).reshape(128, KD * gw * 128))
            f0 += gw
        w1r = np.concatenate([p.reshape(-1) for p in parts])  # flat group blocks
        w2r = (
            W2[e]
            .reshape(NP, W2P, 128, D)
            .transpose(0, 2, 1, 3)
            .reshape(NP, 128, W2P * D)
        )
        in_maps.append(
            {
                "xA": np.ascontiguousarray(xTr[:, : HX * C]).astype(np_io, copy=False),
                "xB": np.ascontiguousarray(xTr[:, HX * C :]).astype(np_io, copy=False),
                "w1": np.ascontiguousarray(w1r).astype(np_io, copy=False),
                "w2": np.ascontiguousarray(w2r).astype(np_io, copy=False),
                "b1r": np.ascontiguousarray(b1[e].reshape(KF, 128).T),
            }
        )

    res = run_bass_kernel_spmd(nc, in_maps, list(range(N_CORES)))

    out = np.zeros_like(x)
    for e in range(E):
        ids = order[starts[e] : starts[e + 1]]
        if ids.size == 0:
            continue
        yr = np.concatenate(
            [
                np.asarray(res.results[e]["yA"], dtype=np.float32),
                np.asarray(res.results[e]["yB"], dtype=np.float32),
            ],
            axis=1,
        )  # [128, KD*C]
        y = yr.reshape(128, KD, C).transpose(2, 1, 0).reshape(C, D)[: ids.size]
        out[ids] = gate[ids, None] * (y + b2[e])
    return out.reshape(orig_shape)



# revision 2
# speedup vs baseline: 1.0048x; 1.0048x over previous
"""MoE top-1 feed-forward (DeepSpeed-style) on 8 Trainium2 NeuronCores.

Strategy (expert parallelism, per the sharding hint):
  - Host computes the (tiny) gate: logits = x @ Wg, softmax, top-1 expert id
    and gate prob per token (float64 for a faithful argmax).
  - Core e holds W1[e]/b1[e]/W2[e]; tokens routed to expert e are dispatched
    to core e, padded to a fixed capacity C=256 so all 8 cores run one SPMD
    program.  Tokens beyond capacity (~2% for the target batch) are computed
    exactly on the host (standard capacity-limited MoE dispatch, but with a
    host fixup instead of drops so the result is exact).
  - Each core runs the dense FFN for its tokens with tokens on the moving
    (free) dimension so no transposes are needed anywhere:
        hT = silu(W1^T @ xT + b1);  yT = W2^T @ hT
  - Weights are packed as flat 128x128 blocks in PE consumption order and
    streamed over the three DMA-initiating rings (SP / ACT / Pool) with a
    greedy earliest-completion schedule against measured queue rates, so
    every tile lands just before the PE consumes it.
  - The PE is kept busy from ~3.5us with warmup matmuls on a scratch tile:
    the HAM clock gate unthrottles (K=8/8, 2.4 GHz) after ~3.4us of
    sustained activity, so the real matmuls start warm instead of paying
    the 1.2 GHz cold ramp.  ACT activation tables are preloaded the same
    way (dummy silu/copy) during the DMA dead time.
  - mm2 trails mm1 by DEFER chunks (PSUM-resident y accumulators) to ride
    out W2 arrival jitter; the tail interleaves PSUM evacuation (DVE+ACT,
    casting to bf16) with the final matmuls and streams y out over three
    rings as soon as each slice is ready.
  - Host combines: out[token] = gate * (y + b2[expert]).
"""

import os
import sys

import numpy as np

try:
    import concourse.mybir as mybir  # noqa: F401
except ModuleNotFoundError:  # fallback if the site hooks aren't installed
    sys.path.insert(0, "/opt/trn_rl_repo")

import concourse.mybir as mybir
import concourse.tile as tile
from concourse import bacc
from concourse.bass_utils import run_bass_kernel_spmd

N_CORES = 8

# Token capacity per core. 256 balances PE time against the ~30us weight
# stream; overflow tokens (2% at the target batch) are fixed up on host.
CAP = int(os.environ.get("BASS_MOE_C", "256"))
DEFER = int(os.environ.get("BASS_MOE_DEFER", "4"))  # mm2 lag in f-chunks
NWARM = int(os.environ.get("BASS_MOE_NWARM", "76"))  # PE warmup matmuls

_CACHE: dict = {}


def _block_groups(nblocks):
    """Per-tile block counts for the weight stream: small leading tiles so
    the PE can start as soon as ~200KB lands, then 16-block tiles whose
    4KB partition lines keep the packet-rate-bound rings at full speed."""
    lead = [6, 6, 6, 14]
    rem = nblocks - sum(lead)
    groups = list(lead)
    while rem > 0:
        w = min(16, rem)
        groups.append(w)
        rem -= w
    return groups


def _schedule(w1_groups, w2_groups, chunk_us, t0_us):
    """Greedy earliest-finish assignment of weight tiles to the three DMA
    rings. Returns {queue: [(kind, tile_idx), ...]} in issue order.
    Rates/starts are HW-measured (GB/s == KB/us)."""
    q = {
        "sync": {"clock": 7.9, "rate": 105.0},
        "act": {"clock": 8.4, "rate": 85.0},
        "pool": {"clock": 8.6, "rate": 135.0},
    }
    # x + b1 are fixed: xa first on sync, b1+xb first on act.
    q["sync"]["clock"] += (3 * CAP * 128 * 2) / 1024 / q["sync"]["rate"]
    q["act"]["clock"] += (3 * CAP * 128 * 2 + 128 * 24 * 4) / 1024 / q["act"]["rate"]

    items = []  # (deadline_us, size_kb, kind, idx)
    o = 0
    for i, g in enumerate(w1_groups):
        items.append((t0_us + chunk_us * (o // 6), g * 32.0, "g", i))
        o += g
    o = 0
    for i, g in enumerate(w2_groups):
        items.append((t0_us + chunk_us * (o // 6 + DEFER), g * 32.0, "p", i))
        o += g
    items.sort(key=lambda it: it[0])

    sched = {"sync": [], "act": [], "pool": []}
    for dl, kb, kind, idx in items:
        best, best_t = None, None
        for name, st in q.items():
            t = st["clock"] + kb / st["rate"]
            if best_t is None or t < best_t:
                best, best_t = name, t
        q[best]["clock"] = best_t
        sched[best].append((kind, idx))
    return sched


def _build_bass(C, D, F):
    f32 = mybir.dt.float32
    dt_io = mybir.dt.bfloat16

    KD, KF = D // 128, F // 128
    NB = KD * KF  # 128x128 blocks per weight matrix
    GR1 = _block_groups(NB)
    GR2 = [16] * (NB // 16) if NB % 16 == 0 else _block_groups(NB)
    assert 256 <= C <= 512 and C % 2 == 0

    # block -> (tile idx, offset within tile), per weight matrix
    def block_map(groups):
        m, t, off = {}, 0, 0
        o = 0
        for t, g in enumerate(groups):
            for j in range(g):
                m[o + j] = (t, j)
            o += g
        return m

    bm1, bm2 = block_map(GR1), block_map(GR2)

    nc = bacc.Bacc(None, target_bir_lowering=False, debug=False)
    # Host-packed images (see kernel() for the packing):
    #   xA/xB [128, 3*C]      col d*C+t = x^T[d*128+p, t], d in 0..2 / 3..5
    #   w1    [NB*128*128]    flat tiles; tile t = blocks b=f*KD+d in
    #                         consumption order, [128, g*128] partition-major
    #   w2    [NB*128*128]    same layout, blocks b=f*KD+dd
    #   b1r   [128, KF]       b1[f*128+p] at [p, f]
    #   yA/yB/yC [128, 2*C]   output yT d-blocks (0,1) / (2,3) / (4,5)
    xA = nc.dram_tensor("xA", [128, 3 * C], dt_io, kind="ExternalInput")
    xB = nc.dram_tensor("xB", [128, 3 * C], dt_io, kind="ExternalInput")
    w1 = nc.dram_tensor("w1", [NB * 128 * 128], dt_io, kind="ExternalInput")
    w2 = nc.dram_tensor("w2", [NB * 128 * 128], dt_io, kind="ExternalInput")
    b1r = nc.dram_tensor("b1r", [128, KF], f32, kind="ExternalInput")
    yA = nc.dram_tensor("yA", [128, 2 * C], dt_io, kind="ExternalOutput")
    yB = nc.dram_tensor("yB", [128, 2 * C], dt_io, kind="ExternalOutput")
    yC = nc.dram_tensor("yC", [128, 2 * C], dt_io, kind="ExternalOutput")

    silu = mybir.ActivationFunctionType.Silu

    # PE pace: ~(C/2.4 + 18)ns per matmul, 12 matmuls per f-chunk
    chunk_us = 12 * (C / 2.4 + 18) / 1000.0
    sched = _schedule(GR1, GR2, chunk_us, 9.8)

    with tile.TileContext(nc) as tc:
        with (
            tc.tile_pool(name="sp", bufs=1) as sp,  # static: x, weights, b1, y
            tc.tile_pool(name="hp", bufs=8) as hp,
            tc.tile_pool(name="ps_h", bufs=2, space="PSUM") as ps_h,
            tc.tile_pool(name="ps_y", bufs=1, space="PSUM") as ps_y,
        ):
            # ---- tiles ----
            b1t = sp.tile([128, KF], f32, tag="b1", name="b1t")
            xa = sp.tile([128, 3 * C], dt_io, tag="xa", name="xa")
            xb = sp.tile([128, 3 * C], dt_io, tag="xb", name="xb")
            warm = sp.tile([128, 128], dt_io, tag="warm", name="warm")
            wsc = sp.tile([128, 4], f32, tag="wsc", name="wsc")
            w1t = [
                sp.tile([128, g * 128], dt_io, tag=f"w1_{t}", name=f"w1t{t}")
                for t, g in enumerate(GR1)
            ]
            w2t = [
                sp.tile([128, g * 128], dt_io, tag=f"w2_{t}", name=f"w2t{t}")
                for t, g in enumerate(GR2)
            ]
            yt = sp.tile([128, KD * C], dt_io, tag="yt", name="yt")
            py = [
                ps_y.tile([128, C], f32, tag=f"y{dd}", name=f"py{dd}")
                for dd in range(KD)
            ]
            phw = ps_h.tile([128, C], f32, tag="hps", name="phw")  # warmup dump

            w1_offs, w2_offs = [], []
            o = 0
            for g in GR1:
                w1_offs.append(o)
                o += g
            o = 0
            for g in GR2:
                w2_offs.append(o)
                o += g

            def load_w(eng, kind, t):
                src, tiles, offs, grs = (
                    (w1, w1t, w1_offs, GR1) if kind == "g" else (w2, w2t, w2_offs, GR2)
                )
                o = offs[t] * 128 * 128
                n = grs[t] * 128 * 128
                eng.dma_start(
                    out=tiles[t][:],
                    in_=src[o : o + n].rearrange("(p w) -> p w", p=128),
                )

            # ---- warmup: DVE memset feeds ~NWARM junk matmuls that keep the
            # PE busy from ~3.5us so HAM unthrottles before real work; ACT
            # preloads its Silu/Copy tables after its DMA issues go out.
            nc.vector.memset(warm[:], 0.0)

            # ---- DMA issue blocks (per-engine program order == ring order)
            nc.sync.dma_start(out=xa[:], in_=xA[:])
            for kind, t in sched["sync"]:
                load_w(nc.sync, kind, t)
            nc.scalar.dma_start(out=b1t[:], in_=b1r[:])
            nc.scalar.dma_start(out=xb[:], in_=xB[:])
            for kind, t in sched["act"]:
                load_w(nc.scalar, kind, t)
            for kind, t in sched["pool"]:
                load_w(nc.gpsimd, kind, t)

            # ACT table preloads (after issues, before first real silu)
            nc.scalar.activation(wsc[:, 0:1], warm[:, 0:1], silu, bias=b1t[:, 0:1])
            nc.scalar.copy(wsc[:, 1:2], wsc[:, 0:1])

            # PE warmup stream
            for _ in range(NWARM):
                nc.tensor.matmul(phw[:, :128], warm[:], warm[:], start=True, stop=True)

            def xsl(d):
                return xa[:, d * C : (d + 1) * C] if d < 3 else xb[:, (d - 3) * C : (d - 2) * C]

            def w1sl(f, d):
                t, j = bm1[f * KD + d]
                return w1t[t][:, j * 128 : (j + 1) * 128]

            def w2sl(f, dd):
                t, j = bm2[f * KD + dd]
                return w2t[t][:, j * 128 : (j + 1) * 128]

            def emit_mm2(f, ht, last=False):
                for dd in range(KD):
                    nc.tensor.matmul(
                        py[dd][:],
                        w2sl(f, dd),
                        ht[:],
                        start=(f == 0),
                        stop=(f == KF - 1),
                    )
                    if last:
                        # stagger PSUM evacuation behind the final matmuls
                        dst = yt[:, dd * C : (dd + 1) * C]
                        if dd < 3:
                            nc.vector.tensor_copy(dst, py[dd][:])
                        else:
                            nc.scalar.copy(dst, py[dd][:])
                        if dd == 1:
                            nc.sync.dma_start(out=yA[:], in_=yt[:, 0 : 2 * C])
                        elif dd == 3:
                            nc.sync.dma_start(out=yB[:], in_=yt[:, 2 * C : 4 * C])
                        elif dd == 5:
                            nc.gpsimd.dma_start(out=yC[:], in_=yt[:, 4 * C : 6 * C])

            pend: list = []
            for f in range(KF):
                ph = ps_h.tile([128, C], f32, tag="hps", name="ph")
                for d in range(KD):
                    nc.tensor.matmul(
                        ph[:], w1sl(f, d), xsl(d), start=(d == 0), stop=(d == KD - 1)
                    )
                ht = hp.tile([128, C], dt_io, tag="ht", name="ht")
                nc.scalar.activation(ht[:], ph[:], silu, bias=b1t[:, f : f + 1])
                pend.append((f, ht))
                if len(pend) > DEFER:
                    emit_mm2(*pend.pop(0))
            while pend:
                f, ht = pend.pop(0)
                emit_mm2(f, ht, last=(f == KF - 1))

    nc.compile()
    return nc


def _get_bass(C, D, F):
    key = (C, D, F, DEFER, NWARM)
    if key not in _CACHE:
        _CACHE[key] = _build_bass(C, D, F)
    return _CACHE[key]


def _gate_host(x, Wg):
    """Top-1 gating in float64: returns (expert_idx [T], gate [T] f32)."""
    logits = x.astype(np.float64) @ Wg.astype(np.float64)
    m = logits.max(-1, keepdims=True)
    p = np.exp(logits - m)
    p /= p.sum(-1, keepdims=True)
    return p.argmax(-1), p.max(-1).astype(np.float32)


def _ffn_host(x, W1e, b1e, W2e, b2e):
    h = x @ W1e + b1e
    h = h * (1.0 / (1.0 + np.exp(-h)))
    return h @ W2e + b2e


def _kernel_numpy(x, Wg, W1, b1, W2, b2):
    """Reference-equivalent fallback (host only)."""
    idx, gate = _gate_host(x, Wg)
    out = np.zeros_like(x)
    for e in range(W1.shape[0]):
        ids = np.nonzero(idx == e)[0]
        if ids.size == 0:
            continue
        out[ids] = gate[ids, None] * _ffn_host(x[ids], W1[e], b1[e], W2[e], b2[e])
    return out


def kernel(hidden_states, Wg, W1, b1, W2, b2):
    hidden_states = np.asarray(hidden_states)
    Wg = np.asarray(Wg, dtype=np.float32)
    W1 = np.asarray(W1, dtype=np.float32)
    b1 = np.asarray(b1, dtype=np.float32)
    W2 = np.asarray(W2, dtype=np.float32)
    b2 = np.asarray(b2, dtype=np.float32)

    orig_shape = hidden_states.shape
    D = orig_shape[-1]
    x = np.ascontiguousarray(hidden_states, dtype=np.float32).reshape(-1, D)
    E, _, F = W1.shape
    KD, KF = D // 128, F // 128

    if E != N_CORES or D % 128 != 0 or F % 128 != 0:
        return _kernel_numpy(x, Wg, W1, b1, W2, b2).reshape(orig_shape)

    C = CAP
    idx, gate = _gate_host(x, Wg)
    order = np.argsort(idx, kind="stable")
    counts = np.bincount(idx, minlength=E)
    starts = np.concatenate([[0], np.cumsum(counts)])

    import ml_dtypes

    np_io = ml_dtypes.bfloat16
    nc = _get_bass(C, D, F)

    GR1 = _block_groups(KD * KF)
    GR2 = [16] * ((KD * KF) // 16) if (KD * KF) % 16 == 0 else _block_groups(KD * KF)

    def pack_blocks(mat, groups, f_major_rows):
        # mat: [D, F] (W1, block b=f*KD+d) or [F, D] (W2, block b=f*KD+dd)
        if f_major_rows:
            blk = mat.reshape(KF, 128, KD, 128)  # [f, p, dd, c]
            blks = blk.transpose(0, 2, 1, 3).reshape(KD * KF, 128, 128)
        else:
            blk = mat.reshape(KD, 128, KF, 128)  # [d, p, f, c]
            blks = blk.transpose(2, 0, 1, 3).reshape(KD * KF, 128, 128)
        parts = []
        o = 0
        for g in groups:
            t = blks[o : o + g]  # [g, 128, 128]
            parts.append(t.transpose(1, 0, 2).reshape(-1))  # [128, g*128] flat
            o += g
        return np.concatenate(parts)

    in_maps = []
    keep_ids, over_ids = [], []
    for e in range(E):
        ids = order[starts[e] : starts[e + 1]]
        keep = ids[:C]
        keep_ids.append(keep)
        over_ids.append(ids[C:])
        xe = np.zeros((C, D), dtype=np.float32)
        xe[: keep.size] = x[keep]
        xTr = xe.reshape(C, KD, 128).transpose(2, 1, 0).reshape(128, KD * C)
        in_maps.append(
            {
                "xA": np.ascontiguousarray(xTr[:, : 3 * C]).astype(np_io, copy=False),
                "xB": np.ascontiguousarray(xTr[:, 3 * C :]).astype(np_io, copy=False),
                "w1": pack_blocks(W1[e], GR1, False).astype(np_io, copy=False),
                "w2": pack_blocks(W2[e], GR2, True).astype(np_io, copy=False),
                "b1r": np.ascontiguousarray(b1[e].reshape(KF, 128).T),
            }
        )

    res = run_bass_kernel_spmd(nc, in_maps, list(range(N_CORES)))

    out = np.zeros_like(x)
    for e in range(E):
        keep = keep_ids[e]
        if keep.size:
            yr = np.concatenate(
                [
                    np.asarray(res.results[e]["yA"], dtype=np.float32),
                    np.asarray(res.results[e]["yB"], dtype=np.float32),
                    np.asarray(res.results[e]["yC"], dtype=np.float32),
                ],
                axis=1,
            )  # [128, KD*C]
            y = yr.reshape(128, KD, C).transpose(2, 1, 0).reshape(C, D)[: keep.size]
            out[keep] = gate[keep, None] * (y + b2[e])
        ov = over_ids[e]
        if ov.size:
            out[ov] = gate[ov, None] * _ffn_host(x[ov], W1[e], b1[e], W2[e], b2[e])
    return out.reshape(orig_shape)


# revision 12
# speedup vs baseline: 1.0062x; 1.0014x over previous
"""MoE top-1 feed-forward (DeepSpeed-style) on 8 Trainium2 NeuronCores.

Strategy (expert parallelism, per the sharding hint):
  - Host computes the (tiny) gate: logits = x @ Wg, softmax, top-1 expert id
    and gate prob per token (float64 for a faithful argmax).
  - Core e holds W1[e]/b1[e]/W2[e]; tokens routed to expert e are dispatched
    to core e, padded to a fixed capacity C=256 so all 8 cores run one SPMD
    program.  Tokens beyond capacity (~2% for the target batch) are computed
    exactly on the host (standard capacity-limited MoE dispatch, but with a
    host fixup instead of drops so the result is exact).
  - Each core runs the dense FFN for its tokens with tokens on the moving
    (free) dimension so no transposes are needed anywhere:
        hT = silu(W1^T @ xT + b1);  yT = W2^T @ hT
  - Weights are packed as flat 128x128 blocks in PE consumption order and
    streamed over the three DMA-initiating rings (SP / ACT / Pool) with a
    greedy earliest-completion schedule against measured queue rates, so
    every tile lands just before the PE consumes it.
  - The PE is kept busy from ~3.5us with warmup matmuls on a scratch tile:
    the HAM clock gate unthrottles (K=8/8, 2.4 GHz) after ~3.4us of
    sustained activity, so the real matmuls start warm instead of paying
    the 1.2 GHz cold ramp.  ACT activation tables are preloaded the same
    way (dummy silu/copy) during the DMA dead time.
  - mm2 trails mm1 by DEFER chunks (PSUM-resident y accumulators) to ride
    out W2 arrival jitter; the tail interleaves PSUM evacuation (DVE+ACT,
    casting to bf16) with the final matmuls and streams y out over three
    rings as soon as each slice is ready.
  - Host combines: out[token] = gate * (y + b2[expert]).
"""

import os
import sys

import numpy as np

try:
    import concourse.mybir as mybir  # noqa: F401
except ModuleNotFoundError:  # fallback if the site hooks aren't installed
    sys.path.insert(0, "/opt/trn_rl_repo")

import concourse.mybir as mybir
import concourse.tile as tile
from concourse import bacc
from concourse.bass_utils import run_bass_kernel_spmd

N_CORES = 8

# Token capacity per core. 256 balances PE time against the ~30us weight
# stream; overflow tokens (2% at the target batch) are fixed up on host.
CAP = int(os.environ.get("BASS_MOE_C", "256"))
DEFER = int(os.environ.get("BASS_MOE_DEFER", "6"))  # mm2 lag in f-chunks
NWARM = int(os.environ.get("BASS_MOE_NWARM", "12"))  # PE warmup matmuls (N=512)

_CACHE: dict = {}


def _block_groups(nblocks):
    """Per-tile block counts for the weight stream: 6-block (one f-chunk)
    leading tiles so the PE can start as soon as ~200KB lands, then 16-block
    tiles (4KB partition lines) for the steady state."""
    lead = [6, 6, 6, 6, 8]
    rem = nblocks - sum(lead)
    groups = list(lead)
    while rem > 0:
        w = min(16, rem)
        groups.append(w)
        rem -= w
    return groups


def _schedule(w1_groups, w2_groups, chunk_us, t0_us, kd):
    """Greedy earliest-finish assignment of weight tiles to the three DMA
    rings. Returns {queue: [(kind, tile_idx), ...]} in issue order.
    Rates (GB/s == KB/us) and queue-start times are HW-measured; the first
    items are pinned: xa+b1 head the sync ring, w1 tile0 + xb head the pool
    ring, and ACT joins late (its stream starts behind the 2.6us of
    activation-table loads).
    """
    kb_x = (kd // 2) * CAP * 128 * 2 / 1024.0
    q = {
        "sync": {"clock": 8.8 + (kb_x + 12.0) / 100.0, "rate": 100.0},
        "act": {"clock": 10.7, "rate": 85.0},
        "pool": {"clock": 9.4 + (w1_groups[0] * 32.0 + kb_x) / 135.0, "rate": 135.0},
    }

    items = []  # (deadline_us, size_kb, kind, idx)
    o = 0
    for i, g in enumerate(w1_groups):
        if i > 0:  # tile 0 pinned on pool
            items.append((t0_us + chunk_us * (o // kd), g * 32.0, "g", i))
        o += g
    o = 0
    for i, g in enumerate(w2_groups):
        items.append((t0_us + chunk_us * (o // kd + DEFER), g * 32.0, "p", i))
        o += g
    items.sort(key=lambda it: it[0])

    sched = {"sync": [], "act": [], "pool": []}
    report = []
    for dl, kb, kind, idx in items:
        best, best_t = None, None
        for name, st in q.items():
            t = st["clock"] + kb / st["rate"]
            if best_t is None or t < best_t:
                best, best_t = name, t
        q[best]["clock"] = best_t
        sched[best].append((kind, idx))
        report.append((kind, idx, best, round(best_t, 1), round(dl, 1)))
    if os.environ.get("BASS_MOE_DEBUG"):
        for r in report:
            slack = r[4] - r[3]
            print(f"  {r[0]}{r[1]:<3d} -> {r[2]:5s} eta={r[3]:5.1f} dl={r[4]:5.1f} "
                  f"slack={slack:+.1f}{'  LATE' if slack < 0 else ''}")
    return sched


def _build_bass(C, D, F):
    f32 = mybir.dt.float32
    dt_io = mybir.dt.bfloat16

    KD, KF = D // 128, F // 128
    NB = KD * KF  # 128x128 blocks per weight matrix
    GR1 = _block_groups(NB)
    GR2 = [16] * (NB // 16) if NB % 16 == 0 else _block_groups(NB)
    assert 256 <= C <= 512 and C % 2 == 0

    # block -> (tile idx, offset within tile), per weight matrix
    def block_map(groups):
        m, t, off = {}, 0, 0
        o = 0
        for t, g in enumerate(groups):
            for j in range(g):
                m[o + j] = (t, j)
            o += g
        return m

    bm1, bm2 = block_map(GR1), block_map(GR2)

    nc = bacc.Bacc(None, target_bir_lowering=False, debug=False)
    # Host-packed images (see kernel() for the packing):
    #   xA/xB [128, 3*C]      col d*C+t = x^T[d*128+p, t], d in 0..2 / 3..5
    #   w1    [NB*128*128]    flat tiles; tile t = blocks b=f*KD+d in
    #                         consumption order, [128, g*128] partition-major
    #   w2    [NB*128*128]    same layout, blocks b=f*KD+dd
    #   b1r   [128, KF]       b1[f*128+p] at [p, f]
    #   yA/yB/yC [128, 2*C]   output yT d-blocks (0,1) / (2,3) / (4,5)
    xA = nc.dram_tensor("xA", [128, 3 * C], dt_io, kind="ExternalInput")
    xB = nc.dram_tensor("xB", [128, 3 * C], dt_io, kind="ExternalInput")
    w1 = nc.dram_tensor("w1", [NB * 128 * 128], dt_io, kind="ExternalInput")
    w2 = nc.dram_tensor("w2", [NB * 128 * 128], dt_io, kind="ExternalInput")
    b1r = nc.dram_tensor("b1r", [128, KF], f32, kind="ExternalInput")
    yA = nc.dram_tensor("yA", [128, 2 * C], dt_io, kind="ExternalOutput")
    yB = nc.dram_tensor("yB", [128, 2 * C], dt_io, kind="ExternalOutput")
    yC = nc.dram_tensor("yC", [128, 2 * C], dt_io, kind="ExternalOutput")

    silu = mybir.ActivationFunctionType.Silu

    # PE pace: ~(C/2.4 + 2.5)ns per matmul, 12 matmuls per f-chunk
    chunk_us = 2 * KD * (C / 2.4 + 2.5) / 1000.0
    sched = _schedule(GR1, GR2, chunk_us, 11.5, KD)

    with tile.TileContext(nc) as tc:
        with (
            tc.tile_pool(name="sp", bufs=1) as sp,  # static: x, weights, b1, y
            tc.tile_pool(name="hp", bufs=8) as hp,
            tc.tile_pool(name="ps_h", bufs=2, space="PSUM") as ps_h,
            tc.tile_pool(name="ps_y", bufs=1, space="PSUM") as ps_y,
        ):
            # ---- tiles ----
            b1t = sp.tile([128, KF], f32, tag="b1", name="b1t")
            xa = sp.tile([128, 3 * C], dt_io, tag="xa", name="xa")
            xb = sp.tile([128, 3 * C], dt_io, tag="xb", name="xb")
            warm = sp.tile([128, 512], dt_io, tag="warm", name="warm")
            wsc = sp.tile([128, 4], f32, tag="wsc", name="wsc")
            wsb = sp.tile([128, 4], dt_io, tag="wsb", name="wsb")
            w1t = [
                sp.tile([128, g * 128], dt_io, tag=f"w1_{t}", name=f"w1t{t}")
                for t, g in enumerate(GR1)
            ]
            w2t = [
                sp.tile([128, g * 128], dt_io, tag=f"w2_{t}", name=f"w2t{t}")
                for t, g in enumerate(GR2)
            ]
            yt = sp.tile([128, KD * C], dt_io, tag="yt", name="yt")
            py = [
                ps_y.tile([128, C], f32, tag=f"y{dd}", name=f"py{dd}")
                for dd in range(KD)
            ]
            phw = ps_h.tile([128, 512], f32, tag="hps", name="phw")  # warmup dump

            w1_offs, w2_offs = [], []
            o = 0
            for g in GR1:
                w1_offs.append(o)
                o += g
            o = 0
            for g in GR2:
                w2_offs.append(o)
                o += g

            def load_w(eng, kind, t):
                src, tiles, offs, grs = (
                    (w1, w1t, w1_offs, GR1) if kind == "g" else (w2, w2t, w2_offs, GR2)
                )
                o = offs[t] * 128 * 128
                n = grs[t] * 128 * 128
                eng.dma_start(
                    out=tiles[t][:],
                    in_=src[o : o + n].rearrange("(p w) -> p w", p=128),
                )

            # ---- warmup: DVE memset feeds NWARM junk matmuls (N=512) that
            # keep the PE busy from the post-preamble barrier (~7.2us) so HAM
            # unthrottles to 2.4GHz before the real matmuls start.
            nc.vector.memset(warm[:], 0.0)
            nc.vector.memset(wsc[:], 0.0)

            # ---- DMA issue blocks (per-engine program order == ring order)
            nc.sync.dma_start(out=xa[:], in_=xA[:])
            nc.sync.dma_start(out=b1t[:], in_=b1r[:])
            for kind, t in sched["sync"]:
                load_w(nc.sync, kind, t)
            # ACT leads with table preloads (dtypes exactly matching the real
            # silu/evac-copy so no table miss hits the critical path later);
            # its DMA issues trail the ~2.6us of table loads, so it only gets
            # late-deadline weight tiles.
            nc.scalar.activation(
                wsb[:, 0:1], wsc[:, 0:1], silu, bias=wsc[:, 1:2]
            )
            nc.scalar.copy(wsb[:, 1:2], wsc[:, 2:3])
            for kind, t in sched["act"]:
                load_w(nc.scalar, kind, t)
            load_w(nc.gpsimd, "g", 0)
            nc.gpsimd.dma_start(out=xb[:], in_=xB[:])
            for kind, t in sched["pool"]:
                load_w(nc.gpsimd, kind, t)

            # PE warmup stream
            for _ in range(NWARM):
                nc.tensor.matmul(phw[:], warm[:, :128], warm[:], start=True, stop=True)

            def xsl(d):
                return xa[:, d * C : (d + 1) * C] if d < 3 else xb[:, (d - 3) * C : (d - 2) * C]

            def w1sl(f, d):
                t, j = bm1[f * KD + d]
                return w1t[t][:, j * 128 : (j + 1) * 128]

            def w2sl(f, dd):
                t, j = bm2[f * KD + dd]
                return w2t[t][:, j * 128 : (j + 1) * 128]

            def emit_mm2(f, ht, last=False):
                for dd in range(KD):
                    nc.tensor.matmul(
                        py[dd][:],
                        w2sl(f, dd),
                        ht[:],
                        start=(f == 0),
                        stop=(f == KF - 1),
                    )
                    if last:
                        # stagger PSUM evacuation behind the final matmuls
                        dst = yt[:, dd * C : (dd + 1) * C]
                        if dd < 3:
                            nc.vector.tensor_copy(dst, py[dd][:])
                        else:
                            nc.scalar.copy(dst, py[dd][:])
                        if dd == 1:
                            nc.sync.dma_start(out=yA[:], in_=yt[:, 0 : 2 * C])
                        elif dd == 3:
                            nc.gpsimd.dma_start(out=yB[:], in_=yt[:, 2 * C : 4 * C])
                        elif dd == 5:
                            nc.scalar.dma_start(out=yC[:], in_=yt[:, 4 * C : 6 * C])

            pend: list = []
            for f in range(KF):
                ph = ps_h.tile([128, C], f32, tag="hps", name="ph")
                for d in range(KD):
                    nc.tensor.matmul(
                        ph[:], w1sl(f, d), xsl(d), start=(d == 0), stop=(d == KD - 1)
                    )
                ht = hp.tile([128, C], dt_io, tag="ht", name="ht")
                nc.scalar.activation(ht[:], ph[:], silu, bias=b1t[:, f : f + 1])
                pend.append((f, ht))
                if len(pend) > DEFER:
                    emit_mm2(*pend.pop(0))
            while pend:
                f, ht = pend.pop(0)
                emit_mm2(f, ht, last=(f == KF - 1))

    nc.compile()
    return nc


def _get_bass(C, D, F):
    key = (C, D, F, DEFER, NWARM)
    if key not in _CACHE:
        _CACHE[key] = _build_bass(C, D, F)
    return _CACHE[key]


def _gate_host(x, Wg):
    """Top-1 gating in float64: returns (expert_idx [T], gate [T] f32)."""
    logits = x.astype(np.float64) @ Wg.astype(np.float64)
    m = logits.max(-1, keepdims=True)
    p = np.exp(logits - m)
    p /= p.sum(-1, keepdims=True)
    return p.argmax(-1), p.max(-1).astype(np.float32)


def _ffn_host(x, W1e, b1e, W2e, b2e):
    h = x @ W1e + b1e
    h = h * (1.0 / (1.0 + np.exp(-h)))
    return h @ W2e + b2e


def _kernel_numpy(x, Wg, W1, b1, W2, b2):
    """Reference-equivalent fallback (host only)."""
    idx, gate = _gate_host(x, Wg)
    out = np.zeros_like(x)
    for e in range(W1.shape[0]):
        ids = np.nonzero(idx == e)[0]
        if ids.size == 0:
            continue
        out[ids] = gate[ids, None] * _ffn_host(x[ids], W1[e], b1[e], W2[e], b2[e])
    return out


def kernel(hidden_states, Wg, W1, b1, W2, b2):
    hidden_states = np.asarray(hidden_states)
    Wg = np.asarray(Wg, dtype=np.float32)
    W1 = np.asarray(W1, dtype=np.float32)
    b1 = np.asarray(b1, dtype=np.float32)
    W2 = np.asarray(W2, dtype=np.float32)
    b2 = np.asarray(b2, dtype=np.float32)

    orig_shape = hidden_states.shape
    D = orig_shape[-1]
    x = np.ascontiguousarray(hidden_states, dtype=np.float32).reshape(-1, D)
    E, _, F = W1.shape
    KD, KF = D // 128, F // 128

    if E != N_CORES or D % 128 != 0 or F % 128 != 0:
        return _kernel_numpy(x, Wg, W1, b1, W2, b2).reshape(orig_shape)

    C = CAP
    idx, gate = _gate_host(x, Wg)
    order = np.argsort(idx, kind="stable")
    counts = np.bincount(idx, minlength=E)
    starts = np.concatenate([[0], np.cumsum(counts)])

    import ml_dtypes

    np_io = ml_dtypes.bfloat16
    nc = _get_bass(C, D, F)

    GR1 = _block_groups(KD * KF)
    GR2 = [16] * ((KD * KF) // 16) if (KD * KF) % 16 == 0 else _block_groups(KD * KF)

    def pack_blocks(mat, groups, f_major_rows):
        # mat: [D, F] (W1, block b=f*KD+d) or [F, D] (W2, block b=f*KD+dd)
        if f_major_rows:
            blk = mat.reshape(KF, 128, KD, 128)  # [f, p, dd, c]
            blks = blk.transpose(0, 2, 1, 3).reshape(KD * KF, 128, 128)
        else:
            blk = mat.reshape(KD, 128, KF, 128)  # [d, p, f, c]
            blks = blk.transpose(2, 0, 1, 3).reshape(KD * KF, 128, 128)
        parts = []
        o = 0
        for g in groups:
            t = blks[o : o + g]  # [g, 128, 128]
            parts.append(t.transpose(1, 0, 2).reshape(-1))  # [128, g*128] flat
            o += g
        return np.concatenate(parts)

    in_maps = []
    keep_ids, over_ids = [], []
    for e in range(E):
        ids = order[starts[e] : starts[e + 1]]
        keep = ids[:C]
        keep_ids.append(keep)
        over_ids.append(ids[C:])
        xe = np.zeros((C, D), dtype=np.float32)
        xe[: keep.size] = x[keep]
        xTr = xe.reshape(C, KD, 128).transpose(2, 1, 0).reshape(128, KD * C)
        in_maps.append(
            {
                "xA": np.ascontiguousarray(xTr[:, : 3 * C]).astype(np_io, copy=False),
                "xB": np.ascontiguousarray(xTr[:, 3 * C :]).astype(np_io, copy=False),
                "w1": pack_blocks(W1[e], GR1, False).astype(np_io, copy=False),
                "w2": pack_blocks(W2[e], GR2, True).astype(np_io, copy=False),
                "b1r": np.ascontiguousarray(b1[e].reshape(KF, 128).T),
            }
        )

    res = run_bass_kernel_spmd(nc, in_maps, list(range(N_CORES)))

    out = np.zeros_like(x)
    for e in range(E):
        keep = keep_ids[e]
        if keep.size:
            yr = np.concatenate(
                [
                    np.asarray(res.results[e]["yA"], dtype=np.float32),
                    np.asarray(res.results[e]["yB"], dtype=np.float32),
                    np.asarray(res.results[e]["yC"], dtype=np.float32),
                ],
                axis=1,
            )  # [128, KD*C]
            y = yr.reshape(128, KD, C).transpose(2, 1, 0).reshape(C, D)[: keep.size]
            out[keep] = gate[keep, None] * (y + b2[e])
        ov = over_ids[e]
        if ov.size:
            out[ov] = gate[ov, None] * _ffn_host(x[ov], W1[e], b1[e], W2[e], b2[e])
    return out.reshape(orig_shape)


# revision 15
# speedup vs baseline: 1.0067x; 1.0005x over previous
"""MoE top-1 feed-forward (DeepSpeed-style) on 8 Trainium2 NeuronCores.

Strategy (expert parallelism, per the sharding hint):
  - Host computes the (tiny) gate: logits = x @ Wg, softmax, top-1 expert id
    and gate prob per token (float64 for a faithful argmax).
  - Core e holds W1[e]/b1[e]/W2[e]; tokens routed to expert e are dispatched
    to core e, padded to a fixed capacity C=256 so all 8 cores run one SPMD
    program.  Tokens beyond capacity (~2% for the target batch) are computed
    exactly on the host (standard capacity-limited MoE dispatch, but with a
    host fixup instead of drops so the result is exact).
  - Each core runs the dense FFN for its tokens with tokens on the moving
    (free) dimension so no transposes are needed anywhere:
        hT = silu(W1^T @ xT + b1);  yT = W2^T @ hT
  - Weights are packed as flat 128x128 blocks in PE consumption order and
    streamed over the three DMA-initiating rings (SP / ACT / Pool) with a
    greedy earliest-completion schedule against measured queue rates, so
    every tile lands just before the PE consumes it.
  - The PE is kept busy from ~3.5us with warmup matmuls on a scratch tile:
    the HAM clock gate unthrottles (K=8/8, 2.4 GHz) after ~3.4us of
    sustained activity, so the real matmuls start warm instead of paying
    the 1.2 GHz cold ramp.  ACT activation tables are preloaded the same
    way (dummy silu/copy) during the DMA dead time.
  - mm2 trails mm1 by DEFER chunks (PSUM-resident y accumulators) to ride
    out W2 arrival jitter; the tail interleaves PSUM evacuation (DVE+ACT,
    casting to bf16) with the final matmuls and streams y out over three
    rings as soon as each slice is ready.
  - Host combines: out[token] = gate * (y + b2[expert]).
"""

import os
import sys

import numpy as np

try:
    import concourse.mybir as mybir  # noqa: F401
except ModuleNotFoundError:  # fallback if the site hooks aren't installed
    sys.path.insert(0, "/opt/trn_rl_repo")

import concourse.mybir as mybir
import concourse.tile as tile
from concourse import bacc
from concourse.bass_utils import run_bass_kernel_spmd

N_CORES = 8

# Token capacity per core. 256 balances PE time against the ~30us weight
# stream; overflow tokens (2% at the target batch) are fixed up on host.
CAP = int(os.environ.get("BASS_MOE_C", "256"))
DEFER = int(os.environ.get("BASS_MOE_DEFER", "6"))  # mm2 lag in f-chunks
NWARM = int(os.environ.get("BASS_MOE_NWARM", "12"))  # PE warmup matmuls (N=512)

_CACHE: dict = {}


def _block_groups(nblocks):
    """Per-tile block counts for the weight stream: 6-block (one f-chunk)
    leading tiles so the PE can start as soon as ~200KB lands, then 16-block
    tiles (4KB partition lines) for the steady state."""
    lead = [6, 6, 6, 6, 8]
    rem = nblocks - sum(lead)
    groups = list(lead)
    while rem > 0:
        w = min(16, rem)
        groups.append(w)
        rem -= w
    return groups


def _schedule(w1_groups, w2_groups, chunk_us, t0_us, kd):
    """Greedy earliest-finish assignment of weight tiles to the three DMA
    rings. Returns {queue: [(kind, tile_idx), ...]} in issue order.
    Rates (GB/s == KB/us) and queue-start times are HW-measured; the first
    items are pinned: xa+b1 head the sync ring, w1 tile0 + xb head the pool
    ring, and ACT joins late (its stream starts behind the 2.6us of
    activation-table loads).
    """
    kb_x = (kd // 2) * CAP * 128 * 2 / 1024.0
    # Starts/rates measured on HW (all three rings share the ~330GB/s
    # aggregate; pinned heads below are the PE-start critical path).
    q = {
        "sync": {"clock": 9.0 + 2 * kb_x / 100.0, "rate": 100.0},  # xa, xb
        "act": {"clock": 9.8 + w1_groups[0] * 32.0 / 105.0, "rate": 105.0},  # w1t0
        "pool": {"clock": 10.35 + (w1_groups[1] * 32.0 + 12) / 120.0, "rate": 120.0},
    }

    items = []  # (deadline_us, size_kb, kind, idx)
    o = 0
    for i, g in enumerate(w1_groups):
        if i > 1:  # tiles 0/1 pinned on act/pool
            items.append((t0_us + chunk_us * (o // kd), g * 32.0, "g", i))
        o += g
    o = 0
    for i, g in enumerate(w2_groups):
        items.append((t0_us + chunk_us * (o // kd + DEFER), g * 32.0, "p", i))
        o += g
    items.sort(key=lambda it: it[0])

    sched = {"sync": [], "act": [], "pool": []}
    report = []
    for dl, kb, kind, idx in items:
        best, best_t = None, None
        for name, st in q.items():
            t = st["clock"] + kb / st["rate"]
            if best_t is None or t < best_t:
                best, best_t = name, t
        q[best]["clock"] = best_t
        sched[best].append((kind, idx))
        report.append((kind, idx, best, round(best_t, 1), round(dl, 1)))
    if os.environ.get("BASS_MOE_DEBUG"):
        for r in report:
            slack = r[4] - r[3]
            print(f"  {r[0]}{r[1]:<3d} -> {r[2]:5s} eta={r[3]:5.1f} dl={r[4]:5.1f} "
                  f"slack={slack:+.1f}{'  LATE' if slack < 0 else ''}")
    return sched


def _build_bass(C, D, F):
    f32 = mybir.dt.float32
    dt_io = mybir.dt.bfloat16

    KD, KF = D // 128, F // 128
    NB = KD * KF  # 128x128 blocks per weight matrix
    GR1 = _block_groups(NB)
    GR2 = [16] * (NB // 16) if NB % 16 == 0 else _block_groups(NB)
    assert 256 <= C <= 512 and C % 2 == 0

    # block -> (tile idx, offset within tile), per weight matrix
    def block_map(groups):
        m, t, off = {}, 0, 0
        o = 0
        for t, g in enumerate(groups):
            for j in range(g):
                m[o + j] = (t, j)
            o += g
        return m

    bm1, bm2 = block_map(GR1), block_map(GR2)

    nc = bacc.Bacc(None, target_bir_lowering=False, debug=False)
    # Host-packed images (see kernel() for the packing):
    #   xA/xB [128, 3*C]      col d*C+t = x^T[d*128+p, t], d in 0..2 / 3..5
    #   w1    [NB*128*128]    flat tiles; tile t = blocks b=f*KD+d in
    #                         consumption order, [128, g*128] partition-major
    #   w2    [NB*128*128]    same layout, blocks b=f*KD+dd
    #   b1r   [128, KF]       b1[f*128+p] at [p, f]
    #   yA/yB/yC [128, 2*C]   output yT d-blocks (0,1) / (2,3) / (4,5)
    xA = nc.dram_tensor("xA", [128, 3 * C], dt_io, kind="ExternalInput")
    xB = nc.dram_tensor("xB", [128, 3 * C], dt_io, kind="ExternalInput")
    w1 = nc.dram_tensor("w1", [NB * 128 * 128], dt_io, kind="ExternalInput")
    w2 = nc.dram_tensor("w2", [NB * 128 * 128], dt_io, kind="ExternalInput")
    b1r = nc.dram_tensor("b1r", [128, KF], f32, kind="ExternalInput")
    yA = nc.dram_tensor("yA", [128, 2 * C], dt_io, kind="ExternalOutput")
    yB = nc.dram_tensor("yB", [128, 2 * C], dt_io, kind="ExternalOutput")
    yC = nc.dram_tensor("yC", [128, 2 * C], dt_io, kind="ExternalOutput")

    silu = mybir.ActivationFunctionType.Silu

    # PE pace: ~(C/2.4 + 2.5)ns per matmul, 12 matmuls per f-chunk
    chunk_us = 2 * KD * (C / 2.4 + 2.5) / 1000.0
    sched = _schedule(GR1, GR2, chunk_us, 11.5, KD)

    with tile.TileContext(nc) as tc:
        with (
            tc.tile_pool(name="sp", bufs=1) as sp,  # static: x, weights, b1, y
            tc.tile_pool(name="hp", bufs=8) as hp,
            tc.tile_pool(name="ps_h", bufs=2, space="PSUM") as ps_h,
            tc.tile_pool(name="ps_y", bufs=1, space="PSUM") as ps_y,
        ):
            # ---- tiles ----
            b1t = sp.tile([128, KF], f32, tag="b1", name="b1t")
            xa = sp.tile([128, 3 * C], dt_io, tag="xa", name="xa")
            xb = sp.tile([128, 3 * C], dt_io, tag="xb", name="xb")
            warm = sp.tile([128, 512], dt_io, tag="warm", name="warm")
            wsc = sp.tile([128, 4], f32, tag="wsc", name="wsc")
            wsb = sp.tile([128, 4], dt_io, tag="wsb", name="wsb")
            w1t = [
                sp.tile([128, g * 128], dt_io, tag=f"w1_{t}", name=f"w1t{t}")
                for t, g in enumerate(GR1)
            ]
            w2t = [
                sp.tile([128, g * 128], dt_io, tag=f"w2_{t}", name=f"w2t{t}")
                for t, g in enumerate(GR2)
            ]
            yt = sp.tile([128, KD * C], dt_io, tag="yt", name="yt")
            py = [
                ps_y.tile([128, C], f32, tag=f"y{dd}", name=f"py{dd}")
                for dd in range(KD)
            ]
            phw = ps_h.tile([128, 512], f32, tag="hps", name="phw")  # warmup dump

            w1_offs, w2_offs = [], []
            o = 0
            for g in GR1:
                w1_offs.append(o)
                o += g
            o = 0
            for g in GR2:
                w2_offs.append(o)
                o += g

            def load_w(eng, kind, t):
                src, tiles, offs, grs = (
                    (w1, w1t, w1_offs, GR1) if kind == "g" else (w2, w2t, w2_offs, GR2)
                )
                o = offs[t] * 128 * 128
                n = grs[t] * 128 * 128
                eng.dma_start(
                    out=tiles[t][:],
                    in_=src[o : o + n].rearrange("(p w) -> p w", p=128),
                )

            # ---- warmup: DVE memset feeds NWARM junk matmuls (N=512) that
            # keep the PE busy from the post-preamble barrier (~7.2us) so HAM
            # unthrottles to 2.4GHz before the real matmuls start.
            nc.vector.memset(warm[:], 0.0)
            nc.vector.memset(wsc[:], 0.0)

            # ---- DMA issue blocks (per-engine program order == ring order)
            nc.sync.dma_start(out=xa[:], in_=xA[:])
            nc.sync.dma_start(out=xb[:], in_=xB[:])
            for kind, t in sched["sync"]:
                load_w(nc.sync, kind, t)
            # ACT table preloads (dtypes exactly matching the real silu /
            # evac-copy so no table miss hits the critical path later); the
            # Tile scheduler hoists the dep-free DMA issues ahead of them.
            nc.scalar.activation(
                wsb[:, 0:1], wsc[:, 0:1], silu, bias=wsc[:, 1:2]
            )
            nc.scalar.copy(wsb[:, 1:2], wsc[:, 2:3])
            load_w(nc.scalar, "g", 0)
            for kind, t in sched["act"]:
                load_w(nc.scalar, kind, t)
            load_w(nc.gpsimd, "g", 1)
            nc.gpsimd.dma_start(out=b1t[:], in_=b1r[:])
            for kind, t in sched["pool"]:
                load_w(nc.gpsimd, kind, t)

            # PE warmup stream
            for _ in range(NWARM):
                nc.tensor.matmul(phw[:], warm[:, :128], warm[:], start=True, stop=True)

            def xsl(d):
                return xa[:, d * C : (d + 1) * C] if d < 3 else xb[:, (d - 3) * C : (d - 2) * C]

            def w1sl(f, d):
                t, j = bm1[f * KD + d]
                return w1t[t][:, j * 128 : (j + 1) * 128]

            def w2sl(f, dd):
                t, j = bm2[f * KD + dd]
                return w2t[t][:, j * 128 : (j + 1) * 128]

            def emit_mm2(f, ht, last=False):
                for dd in range(KD):
                    nc.tensor.matmul(
                        py[dd][:],
                        w2sl(f, dd),
                        ht[:],
                        start=(f == 0),
                        stop=(f == KF - 1),
                    )
                    if last:
                        # stagger PSUM evacuation behind the final matmuls
                        dst = yt[:, dd * C : (dd + 1) * C]
                        if dd < 3:
                            nc.vector.tensor_copy(dst, py[dd][:])
                        else:
                            nc.scalar.copy(dst, py[dd][:])
                        if dd == 1:
                            nc.sync.dma_start(out=yA[:], in_=yt[:, 0 : 2 * C])
                        elif dd == 3:
                            nc.gpsimd.dma_start(out=yB[:], in_=yt[:, 2 * C : 4 * C])
                        elif dd == 5:
                            nc.scalar.dma_start(out=yC[:], in_=yt[:, 4 * C : 6 * C])

            pend: list = []
            for f in range(KF):
                ph = ps_h.tile([128, C], f32, tag="hps", name="ph")
                for d in range(KD):
                    nc.tensor.matmul(
                        ph[:], w1sl(f, d), xsl(d), start=(d == 0), stop=(d == KD - 1)
                    )
                ht = hp.tile([128, C], dt_io, tag="ht", name="ht")
                nc.scalar.activation(ht[:], ph[:], silu, bias=b1t[:, f : f + 1])
                pend.append((f, ht))
                # mm2 trails mm1 by DEFER chunks mid-stream (rides out W2
                # arrival jitter), draining to ~2 near the end so the last
                # mm1 isn't followed by a long pure-mm2 tail.
                target = DEFER if f < KF - DEFER + 1 else max(2, KF - 1 - f)
                while len(pend) > target:
                    emit_mm2(*pend.pop(0))
            while pend:
                f, ht = pend.pop(0)
                emit_mm2(f, ht, last=(f == KF - 1))

    nc.compile()
    return nc


def _get_bass(C, D, F):
    key = (C, D, F, DEFER, NWARM)
    if key not in _CACHE:
        _CACHE[key] = _build_bass(C, D, F)
    return _CACHE[key]


def _gate_host(x, Wg):
    """Top-1 gating in float64: returns (expert_idx [T], gate [T] f32)."""
    logits = x.astype(np.float64) @ Wg.astype(np.float64)
    m = logits.max(-1, keepdims=True)
    p = np.exp(logits - m)
    p /= p.sum(-1, keepdims=True)
    return p.argmax(-1), p.max(-1).astype(np.float32)


def _ffn_host(x, W1e, b1e, W2e, b2e):
    h = x @ W1e + b1e
    h = h * (1.0 / (1.0 + np.exp(-h)))
    return h @ W2e + b2e


def _kernel_numpy(x, Wg, W1, b1, W2, b2):
    """Reference-equivalent fallback (host only)."""
    idx, gate = _gate_host(x, Wg)
    out = np.zeros_like(x)
    for e in range(W1.shape[0]):
        ids = np.nonzero(idx == e)[0]
        if ids.size == 0:
            continue
        out[ids] = gate[ids, None] * _ffn_host(x[ids], W1[e], b1[e], W2[e], b2[e])
    return out


def kernel(hidden_states, Wg, W1, b1, W2, b2):
    hidden_states = np.asarray(hidden_states)
    Wg = np.asarray(Wg, dtype=np.float32)
    W1 = np.asarray(W1, dtype=np.float32)
    b1 = np.asarray(b1, dtype=np.float32)
    W2 = np.asarray(W2, dtype=np.float32)
    b2 = np.asarray(b2, dtype=np.float32)

    orig_shape = hidden_states.shape
    D = orig_shape[-1]
    x = np.ascontiguousarray(hidden_states, dtype=np.float32).reshape(-1, D)
    E, _, F = W1.shape
    KD, KF = D // 128, F // 128

    if E != N_CORES or D % 128 != 0 or F % 128 != 0:
        return _kernel_numpy(x, Wg, W1, b1, W2, b2).reshape(orig_shape)

    C = CAP
    idx, gate = _gate_host(x, Wg)
    order = np.argsort(idx, kind="stable")
    counts = np.bincount(idx, minlength=E)
    starts = np.concatenate([[0], np.cumsum(counts)])

    import ml_dtypes

    np_io = ml_dtypes.bfloat16
    nc = _get_bass(C, D, F)

    GR1 = _block_groups(KD * KF)
    GR2 = [16] * ((KD * KF) // 16) if (KD * KF) % 16 == 0 else _block_groups(KD * KF)

    def pack_blocks(mat, groups, f_major_rows):
        # mat: [D, F] (W1, block b=f*KD+d) or [F, D] (W2, block b=f*KD+dd)
        if f_major_rows:
            blk = mat.reshape(KF, 128, KD, 128)  # [f, p, dd, c]
            blks = blk.transpose(0, 2, 1, 3).reshape(KD * KF, 128, 128)
        else:
            blk = mat.reshape(KD, 128, KF, 128)  # [d, p, f, c]
            blks = blk.transpose(2, 0, 1, 3).reshape(KD * KF, 128, 128)
        parts = []
        o = 0
        for g in groups:
            t = blks[o : o + g]  # [g, 128, 128]
            parts.append(t.transpose(1, 0, 2).reshape(-1))  # [128, g*128] flat
            o += g
        return np.concatenate(parts)

    in_maps = []
    keep_ids, over_ids = [], []
    for e in range(E):
        ids = order[starts[e] : starts[e + 1]]
        keep = ids[:C]
        keep_ids.append(keep)
        over_ids.append(ids[C:])
        xe = np.zeros((C, D), dtype=np.float32)
        xe[: keep.size] = x[keep]
        xTr = xe.reshape(C, KD, 128).transpose(2, 1, 0).reshape(128, KD * C)
        in_maps.append(
            {
                "xA": np.ascontiguousarray(xTr[:, : 3 * C]).astype(np_io, copy=False),
                "xB": np.ascontiguousarray(xTr[:, 3 * C :]).astype(np_io, copy=False),
                "w1": pack_blocks(W1[e], GR1, False).astype(np_io, copy=False),
                "w2": pack_blocks(W2[e], GR2, True).astype(np_io, copy=False),
                "b1r": np.ascontiguousarray(b1[e].reshape(KF, 128).T),
            }
        )

    res = run_bass_kernel_spmd(nc, in_maps, list(range(N_CORES)))

    out = np.zeros_like(x)
    for e in range(E):
        keep = keep_ids[e]
        if keep.size:
            yr = np.concatenate(
                [
                    np.asarray(res.results[e]["yA"], dtype=np.float32),
                    np.asarray(res.results[e]["yB"], dtype=np.float32),
                    np.asarray(res.results[e]["yC"], dtype=np.float32),
                ],
                axis=1,
            )  # [128, KD*C]
            y = yr.reshape(128, KD, C).transpose(2, 1, 0).reshape(C, D)[: keep.size]
            out[keep] = gate[keep, None] * (y + b2[e])
        ov = over_ids[e]
        if ov.size:
            out[ov] = gate[ov, None] * _ffn_host(x[ov], W1[e], b1[e], W2[e], b2[e])
    return out.reshape(orig_shape)


# revision 27
# speedup vs baseline: 1.0243x; 1.0175x over previous
"""MoE top-1 feed-forward (DeepSpeed-style) on 8 Trainium2 NeuronCores.

Strategy (expert parallelism, per the sharding hint):
  - Host computes the (tiny) gate: logits = x @ Wg, softmax, top-1 expert id
    and gate prob per token (float64 for a faithful argmax).
  - Core e holds W1[e]/b1[e]/W2[e]; tokens routed to expert e are dispatched
    to core e, padded to a fixed capacity C=256 so all 8 cores run one SPMD
    program.  Tokens beyond capacity (~2% for the target batch) are computed
    exactly on the host (standard capacity-limited MoE dispatch, but with a
    host fixup instead of drops so the result is exact).
  - Each core runs the dense FFN for its tokens with tokens on the moving
    (free) dimension so no transposes are needed anywhere:
        hT = silu(W1^T @ xT + b1);  yT = W2^T @ hT
  - Weights are packed as flat 128x128 blocks in PE consumption order and
    streamed over the three DMA-initiating rings (SP / ACT / Pool) with a
    greedy earliest-completion schedule against measured queue rates, so
    every tile lands just before the PE consumes it.
  - The PE is kept busy from ~3.5us with warmup matmuls on a scratch tile:
    the HAM clock gate unthrottles (K=8/8, 2.4 GHz) after ~3.4us of
    sustained activity, so the real matmuls start warm instead of paying
    the 1.2 GHz cold ramp.  ACT activation tables are preloaded the same
    way (dummy silu/copy) during the DMA dead time.
  - mm2 trails mm1 by DEFER chunks (PSUM-resident y accumulators) to ride
    out W2 arrival jitter; the tail interleaves PSUM evacuation (DVE+ACT,
    casting to bf16) with the final matmuls and streams y out over three
    rings as soon as each slice is ready.
  - Host combines: out[token] = gate * (y + b2[expert]).
"""

import os
import sys

import numpy as np

try:
    import concourse.mybir as mybir  # noqa: F401
except ModuleNotFoundError:  # fallback if the site hooks aren't installed
    sys.path.insert(0, "/opt/trn_rl_repo")

import concourse.mybir as mybir
import concourse.tile as tile
from concourse import bacc
from concourse.bass_utils import run_bass_kernel_spmd

N_CORES = 8

# Token capacity per core. 256 balances PE time against the ~30us weight
# stream; overflow tokens (2% at the target batch) are fixed up on host.
CAP = int(os.environ.get("BASS_MOE_C", "256"))
DEFER = int(os.environ.get("BASS_MOE_DEFER", "6"))  # mm2 lag in f-chunks
NWARM = int(os.environ.get("BASS_MOE_NWARM", "12"))  # PE warmup matmuls (N=512)

_CACHE: dict = {}


def _block_groups(nblocks, kd):
    """Per-tile block counts for W1: two 1-f-chunk leads (fast PE start)
    then uniform 2-chunk segments."""
    groups = [kd, kd]
    rem = nblocks - 2 * kd
    while rem > 0:
        w = min(2 * kd, rem)
        groups.append(w)
        rem -= w
    return groups


def _block_groups2(nblocks, kd):
    """W2 segments: uniform 2-chunk tiles."""
    groups = []
    rem = nblocks
    while rem > 0:
        w = min(2 * kd, rem)
        groups.append(w)
        rem -= w
    return groups


def _schedule(w1_groups, w2_groups, chunk_us, t0_us, kd):
    """Greedy earliest-finish assignment of weight tiles to the three DMA
    rings. Returns ({queue: [(kind, tile_idx), ...]}, act_mid) where
    act_mid[i] = f-chunk after whose silu the i-th ACT tile is issued.

    Rates (GB/s == KB/us) and ring-start times are HW-measured.  Pinned
    heads (the PE-start critical path): xa+xb on sync, W1 tile0 on act,
    W1 tile1 + b1 on pool.  The ACT engine must stay free for the silus
    from ~13.5us on, and its HWDGE ring stalls the engine if more than
    ~3 big DMAs are outstanding - so ACT's remaining tiles are issued one
    at a time from inside the chunk loop (act_mid), which also bounds its
    ring depth.
    """
    kb_x = (kd // 2) * CAP * 128 * 2 / 1024.0
    q = {
        "sync": {"clock": 9.0 + 2 * kb_x / 105.0, "rate": 105.0},  # xa, xb
        "act": {"clock": 9.8 + w1_groups[0] * 32.0 / 105.0, "rate": 105.0},  # w1t0
        "pool": {"clock": 10.35 + (w1_groups[1] * 32.0 + 12) / 135.0, "rate": 135.0},
    }
    # The ACT HWDGE ring fits ~3 outstanding DMAs; its 5th issue's ring
    # stall resolves just as the first silu needs the engine, so ACT gets
    # at most 4 tiles beyond W1 tile0.
    act_cap = 4
    act_n = 0

    items = []  # (deadline_us, size_kb, kind, idx)
    o = 0
    for i, g in enumerate(w1_groups):
        if i > 1:  # tiles 0/1 pinned on act/pool
            items.append((t0_us + chunk_us * (o // kd), g * 32.0, "g", i))
        o += g
    o = 0
    for i, g in enumerate(w2_groups):
        items.append((t0_us + chunk_us * (o // kd + DEFER), g * 32.0, "p", i))
        o += g
    items.sort(key=lambda it: it[0])

    sched = {"sync": [], "act": [], "pool": []}
    act_mid = []
    report = []
    for dl, kb, kind, idx in items:
        best, best_t = None, None
        for name, st in q.items():
            if name == "act" and act_n >= act_cap:
                continue
            t = st["clock"] + kb / st["rate"]
            if best_t is None or t < best_t:
                best, best_t = name, t
        q[best]["clock"] = best_t
        sched[best].append((kind, idx))
        if best == "act":
            act_n += 1
        report.append((kind, idx, best, round(best_t, 1), round(dl, 1)))
    if os.environ.get("BASS_MOE_DEBUG"):
        for r in report:
            slack = r[4] - r[3]
            print(f"  {r[0]}{r[1]:<3d} -> {r[2]:5s} eta={r[3]:5.1f} dl={r[4]:5.1f} "
                  f"slack={slack:+.1f}{'  LATE' if slack < 0 else ''}")
        print({k: v for k, v in q.items()})
    return sched, act_mid


def _build_bass(C, D, F):
    f32 = mybir.dt.float32
    dt_io = mybir.dt.bfloat16

    KD, KF = D // 128, F // 128
    NB = KD * KF  # 128x128 blocks per weight matrix
    GR1 = _block_groups(NB, KD)
    GR2 = _block_groups2(NB, KD)
    assert 224 <= C <= 512 and C % 2 == 0

    # block -> (tile idx, offset within tile), per weight matrix
    def block_map(groups):
        m, t, off = {}, 0, 0
        o = 0
        for t, g in enumerate(groups):
            for j in range(g):
                m[o + j] = (t, j)
            o += g
        return m

    bm1, bm2 = block_map(GR1), block_map(GR2)

    nc = bacc.Bacc(None, target_bir_lowering=False, debug=False)
    # Host-packed images (see kernel() for the packing):
    #   xA/xB [128, 3*C]      col d*C+t = x^T[d*128+p, t], d in 0..2 / 3..5
    #   w1    [NB*128*128]    flat tiles; tile t = blocks b=f*KD+d in
    #                         consumption order, [128, g*128] partition-major
    #   w2    [NB*128*128]    same layout, blocks b=f*KD+dd
    #   b1r   [128, KF]       b1[f*128+p] at [p, f]
    #   yA/yB/yC [128, 2*C]   output yT d-blocks (0,1) / (2,3) / (4,5)
    xA = nc.dram_tensor("xA", [128, 3 * C], dt_io, kind="ExternalInput")
    xB = nc.dram_tensor("xB", [128, 3 * C], dt_io, kind="ExternalInput")
    w1 = nc.dram_tensor("w1", [NB * 128 * 128], dt_io, kind="ExternalInput")
    w2 = nc.dram_tensor("w2", [NB * 128 * 128], dt_io, kind="ExternalInput")
    b1r = nc.dram_tensor("b1r", [128, KF], f32, kind="ExternalInput")
    yA = nc.dram_tensor("yA", [128, 2 * C], dt_io, kind="ExternalOutput")
    yB = nc.dram_tensor("yB", [128, 2 * C], dt_io, kind="ExternalOutput")
    yC = nc.dram_tensor("yC", [128, 2 * C], dt_io, kind="ExternalOutput")

    silu = mybir.ActivationFunctionType.Silu

    # PE pace: ~(C/2.4 + 2.5)ns per matmul, 12 matmuls per f-chunk
    chunk_us = 2 * KD * (C / 2.4 + 2.5) / 1000.0
    sched, act_mid = _schedule(GR1, GR2, chunk_us, 13.5, KD)

    with tile.TileContext(nc) as tc:
        with (
            tc.tile_pool(name="sp", bufs=1) as sp,  # static: x, weights, b1, y
            tc.tile_pool(name="hp", bufs=8) as hp,
            tc.tile_pool(name="ps_h", bufs=2, space="PSUM") as ps_h,
            tc.tile_pool(name="ps_y", bufs=1, space="PSUM") as ps_y,
        ):
            # ---- tiles ----
            b1t = sp.tile([128, KF], f32, tag="b1", name="b1t")
            xa = sp.tile([128, 3 * C], dt_io, tag="xa", name="xa")
            xb = sp.tile([128, 3 * C], dt_io, tag="xb", name="xb")
            warm = sp.tile([128, 512], dt_io, tag="warm", name="warm")
            wsc = sp.tile([128, 4], f32, tag="wsc", name="wsc")
            wsb = sp.tile([128, 4], dt_io, tag="wsb", name="wsb")
            w1t = [
                sp.tile([128, g * 128], dt_io, tag=f"w1_{t}", name=f"w1t{t}")
                for t, g in enumerate(GR1)
            ]
            w2t = [
                sp.tile([128, g * 128], dt_io, tag=f"w2_{t}", name=f"w2t{t}")
                for t, g in enumerate(GR2)
            ]
            yt = sp.tile([128, KD * C], dt_io, tag="yt", name="yt")
            py = [
                ps_y.tile([128, C], f32, tag=f"y{dd}", name=f"py{dd}")
                for dd in range(KD)
            ]
            phw = ps_h.tile([128, 512], f32, tag="hps", name="phw")  # warmup dump

            w1_offs, w2_offs = [], []
            o = 0
            for g in GR1:
                w1_offs.append(o)
                o += g
            o = 0
            for g in GR2:
                w2_offs.append(o)
                o += g

            def load_w(eng, kind, t):
                src, tiles, offs, grs = (
                    (w1, w1t, w1_offs, GR1) if kind == "g" else (w2, w2t, w2_offs, GR2)
                )
                o = offs[t] * 128 * 128
                n = grs[t] * 128 * 128
                eng.dma_start(
                    out=tiles[t][:],
                    in_=src[o : o + n].rearrange("(p w) -> p w", p=128),
                )

            # ---- warmup: DVE memset feeds NWARM junk matmuls (N=512) that
            # keep the PE busy from the post-preamble barrier (~7.2us) so HAM
            # unthrottles to 2.4GHz before the real matmuls start.
            nc.vector.memset(wsc[:], 0.0)
            nc.vector.memset(warm[:], 0.0)

            # ---- DMA issue blocks (per-engine program order == ring order)
            nc.sync.dma_start(out=xa[:], in_=xA[:])
            nc.sync.dma_start(out=xb[:], in_=xB[:])
            for kind, t in sched["sync"]:
                load_w(nc.sync, kind, t)
            # ACT: W1 tile0 first (PE-start critical), then table preloads
            # (dtypes exactly matching the real silu / evac-copy so no table
            # miss hits the critical path later), then its few ring-safe
            # weight tiles.
            load_w(nc.scalar, "g", 0)
            nc.scalar.activation(
                wsb[:, 0:1], wsc[:, 0:1], silu, bias=wsc[:, 1:2]
            )
            nc.scalar.copy(wsb[:, 1:2], wsc[:, 2:3])
            for kind, t in sched["act"]:
                load_w(nc.scalar, kind, t)
            load_w(nc.gpsimd, "g", 1)
            nc.gpsimd.dma_start(out=b1t[:], in_=b1r[:])
            for kind, t in sched["pool"]:
                load_w(nc.gpsimd, kind, t)

            # PE warmup stream
            for _ in range(NWARM):
                nc.tensor.matmul(phw[:], warm[:, :128], warm[:], start=True, stop=True)

            def xsl(d):
                return xa[:, d * C : (d + 1) * C] if d < 3 else xb[:, (d - 3) * C : (d - 2) * C]

            def w1sl(f, d):
                t, j = bm1[f * KD + d]
                return w1t[t][:, j * 128 : (j + 1) * 128]

            def w2sl(f, dd):
                t, j = bm2[f * KD + dd]
                return w2t[t][:, j * 128 : (j + 1) * 128]

            def emit_mm2(f, ht, last=False):
                for dd in range(KD):
                    nc.tensor.matmul(
                        py[dd][:],
                        w2sl(f, dd),
                        ht[:],
                        start=(f == 0),
                        stop=(f == KF - 1),
                    )
                    if last:
                        # stagger PSUM evacuation behind the final matmuls
                        dst = yt[:, dd * C : (dd + 1) * C]
                        if dd < 3:
                            nc.vector.tensor_copy(dst, py[dd][:])
                        else:
                            nc.scalar.copy(dst, py[dd][:])
                        if dd == 1:
                            nc.sync.dma_start(out=yA[:], in_=yt[:, 0 : 2 * C])
                        elif dd == 3:
                            nc.gpsimd.dma_start(out=yB[:], in_=yt[:, 2 * C : 4 * C])
                        elif dd == 5:
                            nc.scalar.dma_start(out=yC[:], in_=yt[:, 4 * C : 6 * C])

            pend: list = []
            for f in range(KF):
                ph = ps_h.tile([128, C], f32, tag="hps", name="ph")
                for d in range(KD):
                    nc.tensor.matmul(
                        ph[:], w1sl(f, d), xsl(d), start=(d == 0), stop=(d == KD - 1)
                    )
                ht = hp.tile([128, C], dt_io, tag="ht", name="ht")
                nc.scalar.activation(ht[:], ph[:], silu, bias=b1t[:, f : f + 1])
                pend.append((f, ht))
                # mm2 trails mm1 by DEFER chunks mid-stream (rides out W2
                # arrival jitter), draining to ~2 near the end so the last
                # mm1 isn't followed by a long pure-mm2 tail.
                target = DEFER if f < KF - DEFER + 1 else max(2, KF - 1 - f)
                while len(pend) > target:
                    emit_mm2(*pend.pop(0))
            while pend:
                f, ht = pend.pop(0)
                emit_mm2(f, ht, last=(f == KF - 1))

    nc.compile()
    return nc


def _get_bass(C, D, F):
    key = (C, D, F, DEFER, NWARM)
    if key not in _CACHE:
        _CACHE[key] = _build_bass(C, D, F)
    return _CACHE[key]


def _gate_host(x, Wg):
    """Top-1 gating in float64: returns (expert_idx [T], gate [T] f32)."""
    logits = x.astype(np.float64) @ Wg.astype(np.float64)
    m = logits.max(-1, keepdims=True)
    p = np.exp(logits - m)
    p /= p.sum(-1, keepdims=True)
    return p.argmax(-1), p.max(-1).astype(np.float32)


def _ffn_host(x, W1e, b1e, W2e, b2e):
    h = x @ W1e + b1e
    h = h * (1.0 / (1.0 + np.exp(-h)))
    return h @ W2e + b2e


def _kernel_numpy(x, Wg, W1, b1, W2, b2):
    """Reference-equivalent fallback (host only)."""
    idx, gate = _gate_host(x, Wg)
    out = np.zeros_like(x)
    for e in range(W1.shape[0]):
        ids = np.nonzero(idx == e)[0]
        if ids.size == 0:
            continue
        out[ids] = gate[ids, None] * _ffn_host(x[ids], W1[e], b1[e], W2[e], b2[e])
    return out


def kernel(hidden_states, Wg, W1, b1, W2, b2):
    hidden_states = np.asarray(hidden_states)
    Wg = np.asarray(Wg, dtype=np.float32)
    W1 = np.asarray(W1, dtype=np.float32)
    b1 = np.asarray(b1, dtype=np.float32)
    W2 = np.asarray(W2, dtype=np.float32)
    b2 = np.asarray(b2, dtype=np.float32)

    orig_shape = hidden_states.shape
    D = orig_shape[-1]
    x = np.ascontiguousarray(hidden_states, dtype=np.float32).reshape(-1, D)
    E, _, F = W1.shape
    KD, KF = D // 128, F // 128

    if E != N_CORES or D % 128 != 0 or F % 128 != 0:
        return _kernel_numpy(x, Wg, W1, b1, W2, b2).reshape(orig_shape)

    C = CAP
    idx, gate = _gate_host(x, Wg)
    order = np.argsort(idx, kind="stable")
    counts = np.bincount(idx, minlength=E)
    starts = np.concatenate([[0], np.cumsum(counts)])

    import ml_dtypes

    np_io = ml_dtypes.bfloat16
    nc = _get_bass(C, D, F)

    GR1 = _block_groups(KD * KF, KD)
    GR2 = _block_groups2(KD * KF, KD)

    def pack_blocks(mat, groups, f_major_rows):
        # mat: [D, F] (W1, block b=f*KD+d) or [F, D] (W2, block b=f*KD+dd)
        if f_major_rows:
            blk = mat.reshape(KF, 128, KD, 128)  # [f, p, dd, c]
            blks = blk.transpose(0, 2, 1, 3).reshape(KD * KF, 128, 128)
        else:
            blk = mat.reshape(KD, 128, KF, 128)  # [d, p, f, c]
            blks = blk.transpose(2, 0, 1, 3).reshape(KD * KF, 128, 128)
        parts = []
        o = 0
        for g in groups:
            t = blks[o : o + g]  # [g, 128, 128]
            parts.append(t.transpose(1, 0, 2).reshape(-1))  # [128, g*128] flat
            o += g
        return np.concatenate(parts)

    in_maps = []
    keep_ids, over_ids = [], []
    for e in range(E):
        ids = order[starts[e] : starts[e + 1]]
        keep = ids[:C]
        keep_ids.append(keep)
        over_ids.append(ids[C:])
        xe = np.zeros((C, D), dtype=np.float32)
        xe[: keep.size] = x[keep]
        xTr = xe.reshape(C, KD, 128).transpose(2, 1, 0).reshape(128, KD * C)
        in_maps.append(
            {
                "xA": np.ascontiguousarray(xTr[:, : 3 * C]).astype(np_io, copy=False),
                "xB": np.ascontiguousarray(xTr[:, 3 * C :]).astype(np_io, copy=False),
                "w1": pack_blocks(W1[e], GR1, False).astype(np_io, copy=False),
                "w2": pack_blocks(W2[e], GR2, True).astype(np_io, copy=False),
                "b1r": np.ascontiguousarray(b1[e].reshape(KF, 128).T),
            }
        )

    res = run_bass_kernel_spmd(nc, in_maps, list(range(N_CORES)))

    out = np.zeros_like(x)
    for e in range(E):
        keep = keep_ids[e]
        if keep.size:
            yr = np.concatenate(
                [
                    np.asarray(res.results[e]["yA"], dtype=np.float32),
                    np.asarray(res.results[e]["yB"], dtype=np.float32),
                    np.asarray(res.results[e]["yC"], dtype=np.float32),
                ],
                axis=1,
            )  # [128, KD*C]
            y = yr.reshape(128, KD, C).transpose(2, 1, 0).reshape(C, D)[: keep.size]
            out[keep] = gate[keep, None] * (y + b2[e])
        ov = over_ids[e]
        if ov.size:
            out[ov] = gate[ov, None] * _ffn_host(x[ov], W1[e], b1[e], W2[e], b2[e])
    return out.reshape(orig_shape)
